# revision 24
# baseline (speedup 1.0000x reference)
"""Chamfer distance kernel for 8 Trainium2 NeuronCores.

Problem: pred (4, 3, 8192), target (4, 3, 8192) ->
  scalar = mean_n min_m ||p_n - t_m||^2 + mean_m min_n ||p_n - t_m||^2
computed per batch then averaged (means over (B,N) and (B,M)).

Sharding: core <-> (b = core//2, n-half = core%2). Each core computes its
4096 x 8192 block of the (negated) squared-distance matrix
  E[n, m] = 2*x.y - |y|^2 - |x|^2  = -D[n, m]
via K=5 TensorE matmuls (weights prepared host-side: lhsT rows
[2x0, 2x1, 2x2, -1, -|x|^2], rhs rows [y0, y1, y2, |y|^2, 1]), then tracks
  colmax[m] = max_n E[n, m]   (DVE tensor_tensor max, PSUM source)
  rowmax[n] = max_m E[n, m]   (DVE tensor_reduce max, PSUM source)
colmax is partition-reduced on device (PE transpose + DVE reduce) so the
output is tiny. Host combines: min = -max, means, final scalar. No
cross-core collectives; 4 row-group-tiled concurrent matmuls (tile_position
strips) keep the PE ahead of the DVE, which is the bottleneck engine at
~2 x 33.5M elements x 1 elem/cycle/lane @ 0.96 GHz ~ 550us per core.
"""

import sys

import numpy as np

if "/opt/trn_rl_repo" not in sys.path:
    sys.path.insert(0, "/opt/trn_rl_repo")

B, C, N, M = 4, 3, 8192, 8192
P = 128          # partitions
CH = 512         # matmul moving-operand chunk (one PSUM bank of fp32)
NSTRIP = 4       # concurrent row-group-tiled matmuls (K=5 each, 32-partition strips)
SCW = NSTRIP * CH  # 2048 columns per PSUM superchunk (4 banks)
NT = (N // 2) // P  # 32 n-tiles per core
NSC = M // SCW      # 4 superchunks covering M
BIG = 3.0e38

# Engine split for the row-max partials: DVE tensor_reduce when
# (t % DVE_ROWMAX_EVERY) == 0, else ScalarE-copy + GPSIMD pool_max.
# GPSIMD/ACT free-dim reductions are rejected by this walrus build, so
# everything runs on DVE (DVE_ROWMAX_EVERY = 1).
DVE_ROWMAX_EVERY = 1
MM_DTYPE = "float32"
REPS = 1  # >1 wraps the compute in a For_i loop for repeat-slope timing
          # (max-accumulation is idempotent, so results are unchanged)
ABLATE = set()  # timing diagnostics: subset of {"mm", "tt", "reduce", "memset"}
FLAT_Y = False  # True: no row-group striping; y weights live at partitions 0-4
                # over the full M and all matmuls run at tile_position (0, 0)

_cache = {}


def _build_program():
    from contextlib import ExitStack

    import concourse.bass as bass
    import concourse.mybir as mybir
    import concourse.tile as tile
    from concourse import bacc
    from concourse.bass import BassVectorEngine

    f32 = mybir.dt.float32
    mm_dt = getattr(mybir.dt, MM_DTYPE)

    nc = bacc.Bacc("TRN2", target_bir_lowering=False, debug=False)
    xw = nc.dram_tensor("xw", [P, NT * P], f32, kind="ExternalInput")
    yw = nc.dram_tensor("yw", [P, M if FLAT_Y else SCW], f32, kind="ExternalInput")
    ident = nc.dram_tensor("ident", [P, P], f32, kind="ExternalInput")
    colmax_o = nc.dram_tensor("colmax", [P, M // P], f32, kind="ExternalOutput")
    rowmax_o = nc.dram_tensor("rowmax", [P, NT], f32, kind="ExternalOutput")

    with tile.TileContext(nc) as tc, ExitStack() as ctx:
        cpool = ctx.enter_context(tc.tile_pool(name="const", bufs=1))
        psum_pool = ctx.enter_context(tc.tile_pool(name="psum", bufs=2, space="PSUM"))
        spool = ctx.enter_context(tc.tile_pool(name="scratch", bufs=3))

        yw_sb = cpool.tile([P, M if FLAT_Y else SCW], mm_dt, tag="yw")
        nc.sync.dma_start(yw_sb[:], yw[:])
        xw_sb = cpool.tile([P, NT * P], mm_dt, tag="xw")
        for q in range(4):
            w = NT * P // 4
            nc.sync.dma_start(xw_sb[:, q * w:(q + 1) * w], xw[:, q * w:(q + 1) * w])

        ident_sb = cpool.tile([P, P], f32, tag="ident")
        nc.sync.dma_start(ident_sb[:], ident[:])
        colmax = cpool.tile([P, M], f32, tag="colmax")
        # row-max partials, separated by producing engine to avoid false deps
        rowp_gp = cpool.tile([P, NT * NSC], f32, tag="rowp_gp")
        rowp_dve = cpool.tile([P, NT * NSC], f32, tag="rowp_dve")
        rowmax = cpool.tile([P, NT], f32, tag="rowmax")

        if "memset" not in ABLATE:
            nc.vector.memset(colmax[:], -BIG)
            nc.vector.memset(rowp_dve[:], -BIG)
            nc.vector.memset(rowp_gp[:], -BIG)

        def compute(_iv=None):
            _emit_compute(nc, tc, mybir, spool, psum_pool, xw_sb, yw_sb,
                          colmax, rowp_gp, rowp_dve, rowmax, colmax_o, rowmax_o,
                          ident_sb)

        if REPS == 1:
            compute()
        else:
            with tc.For_i(0, REPS) as iv:
                compute(iv)

    _fix_pool_aps(nc, mybir)
    nc.compile()
    return nc


def _emit_compute(nc, tc, mybir, spool, psum_pool, xw_sb, yw_sb,
                  colmax, rowp_gp, rowp_dve, rowmax, colmax_o, rowmax_o,
                  ident_sb):
    from concourse.bass import BassVectorEngine

    f32 = mybir.dt.float32
    if True:
        for sc in range(NSC):
            for t in range(NT):
                ps = psum_pool.tile([P, SCW], f32, tag="ps")
                if "mm" not in ABLATE:
                    for s in range(NSTRIP):
                        if FLAT_Y:
                            nc.tensor.matmul(
                                ps[:, s * CH:(s + 1) * CH],
                                lhsT=xw_sb[0:5, t * P:(t + 1) * P],
                                rhs=yw_sb[0:5, sc * SCW + s * CH:sc * SCW + (s + 1) * CH],
                                start=True,
                                stop=True,
                                tile_position=(0, 0),
                            )
                        else:
                            nc.tensor.matmul(
                                ps[:, s * CH:(s + 1) * CH],
                                lhsT=xw_sb[32 * s:32 * s + 5, t * P:(t + 1) * P],
                                rhs=yw_sb[32 * s:32 * s + 5, sc * CH:(sc + 1) * CH],
                                start=True,
                                stop=True,
                                tile_position=(32 * s, 0),
                            )
                cm = colmax[:, sc * SCW:(sc + 1) * SCW]
                if "tt" not in ABLATE:
                    nc.vector.tensor_tensor(cm, ps[:], cm, mybir.AluOpType.max)
                idx = t * NSC + sc
                use_dve = DVE_ROWMAX_EVERY and (t % DVE_ROWMAX_EVERY == 0)
                if "reduce" in ABLATE:
                    pass
                elif use_dve:
                    nc.vector.tensor_reduce(
                        rowp_dve[:, idx:idx + 1], ps[:],
                        axis=mybir.AxisListType.X, op=mybir.AluOpType.max,
                    )
                else:
                    sb = spool.tile([P, SCW], f32, tag="scratch")
                    nc.scalar.copy(sb[:], ps[:])
                    BassVectorEngine.pool(
                        nc.gpsimd, rowp_gp[:, idx:idx + 1], sb[:],
                        func=mybir.PoolFunctionType.max,
                    )
        if "dmaout" not in ABLATE:
            # on-device partition-reduce of colmax: PE transposes 128x128
            # blocks into PSUM, DVE free-dim max-reduces each -> (P, M//P)
            colred = spool.tile([P, M // P], f32, tag="colred")
            for g in range(M // P // 4):
                tps = psum_pool.tile([P, 4 * P], f32, tag="ps")
                for j in range(4):
                    blk = 4 * g + j
                    nc.tensor.transpose(
                        tps[:, j * P:(j + 1) * P],
                        colmax[:, blk * P:(blk + 1) * P],
                        ident_sb[:],
                    )
                for j in range(4):
                    blk = 4 * g + j
                    nc.vector.tensor_reduce(
                        colred[:, blk:blk + 1], tps[:, j * P:(j + 1) * P],
                        axis=mybir.AxisListType.X, op=mybir.AluOpType.max,
                    )
            nc.sync.dma_start(colmax_o[:], colred[:])

        if "finals" not in ABLATE:
            rp = spool.tile([P, NT * NSC], f32, tag="rowcomb")
            nc.vector.tensor_tensor(rp[:], rowp_gp[:], rowp_dve[:], mybir.AluOpType.max)
            nc.vector.tensor_reduce(
                rowmax[:], rp[:].rearrange("p (t s) -> p t s", s=NSC),
                axis=mybir.AxisListType.X, op=mybir.AluOpType.max,
            )
            nc.sync.dma_start(rowmax_o[:], rowmax[:])


def _fix_pool_aps(nc, mybir):
    """TileContext's re-lowering drops pool's required 5d input AP; restore it."""
    from concourse import ap_utils

    for f in nc.m.functions:
        for bb in f.blocks:
            for ins in bb.instructions:
                if type(ins).__name__ == "InstPool":
                    pap = ins.ins[0]
                    nd = len(pap.ap)
                    if nd != 5:
                        new_dims = list(range(1, 6 - nd))
                        pap.ap = mybir.VecI64Pair(
                            ap_utils.expand_dims_ap(pap.ap, new_dims)
                        )


def _get_program():
    if "nc" not in _cache:
        _cache["nc"] = _build_program()
    return _cache["nc"]


def make_core_inputs(x, y):
    """x: (3, 4096) pred shard, y: (3, 8192) target. Returns {xw, yw} fp32."""
    x = np.asarray(x, np.float32)
    y = np.asarray(y, np.float32)
    x2 = (x.astype(np.float64) ** 2).sum(0).astype(np.float32)
    y2 = (y.astype(np.float64) ** 2).sum(0).astype(np.float32)
    xw = np.zeros((P, NT * P), np.float32)
    if FLAT_Y:
        yw = np.zeros((P, M), np.float32)
        for c in range(C):
            yw[c] = y[c]
        yw[3] = y2
        yw[4] = 1.0
    else:
        yw = np.zeros((P, SCW), np.float32)
        # m = sc*SCW + s*CH + u  lives at yw[32*s + k, sc*CH + u]
        yr = y.reshape(C, NSC, NSTRIP, CH)
        y2r = y2.reshape(NSC, NSTRIP, CH)
        for s in range(NSTRIP):
            for c in range(C):
                yw[32 * s + c] = yr[c, :, s, :].reshape(-1)
            yw[32 * s + 3] = y2r[:, s, :].reshape(-1)
            yw[32 * s + 4] = 1.0
    for s in range(NSTRIP if not FLAT_Y else 1):
        for c in range(C):
            xw[32 * s + c] = 2.0 * x[c]
        xw[32 * s + 3] = -1.0
        xw[32 * s + 4] = -x2
    return {"xw": xw, "yw": yw, "ident": np.eye(P, dtype=np.float32)}


def combine_results(results):
    """results: list of 8 dicts with 'rowmax' (P, NT) and 'colmax' (P, M//P).

    rowmax[p, t] = max_m E[n, m] for n = t*P + p (E = -dists); colmax[p, g]
    = max_n E[n, m] for m = g*P + p (already partition-reduced on device).
    """
    row_sum = 0.0
    col_mean_sum = 0.0
    for b in range(B):
        for h in range(2):
            row_sum += -np.float64(results[2 * b + h]["rowmax"]).sum()
        cmx = np.maximum(results[2 * b]["colmax"], results[2 * b + 1]["colmax"])
        col_mean_sum += (-cmx.astype(np.float64)).mean()
    return np.float32(row_sum / (B * N) + col_mean_sum / B)


def kernel(pred, target):
    from concourse.bass_utils import run_bass_kernel_spmd

    pred = np.asarray(pred, np.float32)
    target = np.asarray(target, np.float32)
    nc = _get_program()
    in_maps = []
    for core in range(8):
        b, h = core // 2, core % 2
        in_maps.append(make_core_inputs(pred[b][:, h * (N // 2):(h + 1) * (N // 2)],
                                        target[b]))
    res = run_bass_kernel_spmd(nc, in_maps, list(range(8))).results
    return combine_results(res)
